# revision 9
# baseline (speedup 1.0000x reference)
"""NunchakuFP4GemmOp Trainium2 kernel.

out = fake_quant_fp4(x) @ fake_quant_fp4(weight).T + bias
  x: [8192, 4096] f32, weight: [4096, 4096] f32, bias: [4096] f32

Strategy: data-parallel shard of M (tokens) across 8 NeuronCores. Each core
quantizes its x-shard and the full weight (replicated), runs a bf16 GEMM on
the tensor engine (dequantized NVFP4 values are exact in bf16: e2m1 grid
value x e4m3 scale has <= 6 significand bits), and adds bias on eviction.

Quantization reproduces the reference bit-exactly with fp32 vector ops:
  - per-16-group abs-max via tensor_reduce(apply_absolute_value, axis=X)
  - fp8_e4m3 RNE of amax/6 via a Dekker split (normal range) + magic-add
    (subnormal range), validated exhaustively against ml_dtypes
  - e2m1 rounding via region-selected magic-add, clamp at +-6, in a single
    fused custom DVE op

GEMM layout: out^T tiles [n, m] are computed on device (stationary = wqT
chunks produced by SBUF->SBUF DMA xbar transposes; moving = resident xqT);
the host transposes on gather.
"""

import numpy as np

M_FULL, K_FULL, N_FULL = 8192, 4096, 4096
N_CORES = 8
M_LOC = M_FULL // N_CORES

_PROGRAM_CACHE = {}


# --------------------------------------------------------------------------
# custom DVE ops
# --------------------------------------------------------------------------

def _register_custom_ops():
    """Register the fused quantization ops in the concourse custom-DVE
    registry (the documented extension point is appending to dve_ops.OPS;
    we do it at runtime instead of patching the source file)."""
    from concourse import dve_ops
    from concourse.dve_spec import (
        Spec, Src0, Src1, C0, C1, C2, Zero, One, maxx, minn, select, lower,
    )
    from concourse.dve_uop import DveOpSpec

    F32 = np.float32

    def mk(name, spec, subdim=False):
        for op in dve_ops.OPS:
            if op.name == name:
                return op
        row = dve_ops._CUSTOM_DVE_ROW_BASE + len(dve_ops.OPS)
        assert row < 0x20
        shas = {}
        for ver in ("v3", "v4"):
            s = DveOpSpec(name=name, opcode=row, uops=lower(spec, ver=ver),
                          rd1_en=dve_ops.has_src1(spec))
            shas[ver] = s.sha(ver)
        op = dve_ops.DveOp(name, spec, subdim=subdim, uops_sha=shas)
        dve_ops.OPS.append(op)
        dve_ops.CUSTOM_DVE_SPECS[name] = spec
        dve_ops._SUB_OPCODE_FOR_NAME[name] = row
        return op

    # ---- QUANT_MAGIC: region-selected magic constant from r ----
    # in0 = r fp32.  s0 = 4.0, s1 = 6291456.0 (M1), imm2 = 12582912.0 (2*M1)
    # out m = M1 * (1 + (r^2>=4) + 2*(r^2>=16))  in {M1, 2*M1, 4*M1}
    e16 = C0 * C0
    a2 = Src0 * Src0
    b1 = a2 >= C0
    b2 = a2 >= e16
    u = b1 * C1
    v = b2 * C2
    mbody = (u + v) + C1

    def _magic_ref(in0, in1, s0, s1, imm2):
        r = in0.astype(F32)
        r2 = (r * r).astype(F32)
        u_ = ((r2 >= F32(s0)).astype(F32) * F32(s1)).astype(F32)
        v_ = ((r2 >= (F32(s0) * F32(s0))).astype(F32) * F32(imm2)).astype(F32)
        return ((u_ + v_).astype(F32) + F32(s1)).astype(F32)

    magic = mk("QUANT_MAGIC_NFP4", Spec(body=mbody, reference=_magic_ref))

    # ---- QUANT_ROUND: out = (clip(r, -6, 6) + m) - m ----
    # in0 = r fp32, in1 = m, s0 = 6.0, s1 = -6.0
    rc = maxx(minn(Src0, C0), C1)
    t = rc + Src1
    qbody = t - Src1

    def _round_ref(in0, in1, s0, s1, imm2):
        r = in0.astype(F32)
        m = np.broadcast_to(np.asarray(in1, F32), r.shape).astype(F32)
        rcl = np.maximum(np.minimum(r, F32(s0)), F32(s1)).astype(F32)
        return ((rcl + m).astype(F32) - m).astype(F32)

    quant = mk("QUANT_ROUND_NFP4", Spec(body=qbody, reference=_round_ref))

    # ---- SCALE_DEKKER: out = RNE-to-4-sig-bits(in0 * c0) via Dekker split ----
    # in0 = amax, s0 = 1/6, s1 = 1048577.0 (2^20 + 1)
    t = Src0 * C0
    c = t * C1
    d = c - t
    hi = c - d

    def _dek_ref(in0, in1, s0, s1, imm2):
        tt = (in0.astype(F32) * F32(s0)).astype(F32)
        cc = (tt * F32(s1)).astype(F32)
        dd = (cc - tt).astype(F32)
        return (cc - dd).astype(F32)

    dek = mk("SCALE_DEKKER_NFP4", Spec(body=hi, reference=_dek_ref))

    # ---- SCALE_SUB: fp8 subnormal path + select + safety floor ----
    # in0 = amax, in1 = hi (from SCALE_DEKKER), s0 = 1/6, s1 = 24576.0,
    # imm2 = 0.015625 (2^-6).  out = max(t < 2^-6 ? magic(t) : hi, 2^-12)
    t2 = Src0 * C0
    e2 = t2 + C1
    f2b = e2 - C1
    sel = select(t2 < C2, f2b, Src1)
    eps = C2 * C2  # 2^-12, stream-invariant
    sbody = maxx(sel, eps)

    def _ssub_ref(in0, in1, s0, s1, imm2):
        tt = (in0.astype(F32) * F32(s0)).astype(F32)
        ee = (tt + F32(s1)).astype(F32)
        ff = (ee - F32(s1)).astype(F32)
        r = np.where(tt < F32(imm2), ff, in1.astype(F32)).astype(F32)
        return np.maximum(r, F32(imm2) * F32(imm2)).astype(F32)

    ssub = mk("SCALE_SUB_NFP4", Spec(body=sbody, reference=_ssub_ref))

    return magic, quant, dek, ssub


# --------------------------------------------------------------------------
# kernel program
# --------------------------------------------------------------------------

def build_program(m_loc=M_LOC, k=K_FULL, n=N_FULL):
    """Build (and compile) the per-core Bass program. Same program runs
    SPMD on all 8 cores with different x shards."""
    import concourse.bass as bass
    import concourse.mybir as mybir
    import concourse.tile as tile
    from concourse import bacc

    magic_op, quant_op, dek_op, ssub_op = _register_custom_ops()

    F32, BF16 = mybir.dt.float32, mybir.dt.bfloat16
    P = 128
    G = 16                      # fp4 group size
    KG = k // G                 # groups per row
    KC = k // P                 # 128-wide k-chunks
    MT = m_loc // P             # m tiles (x quant phase)
    NT = n // P                 # n tiles (gemm loop)
    MH = m_loc // 512           # moving-operand slices per matmul row

    nc = bacc.Bacc("TRN2", debug=False)

    x_dram = nc.dram_tensor("x_shard", [m_loc, k], F32, kind="ExternalInput")
    w_dram = nc.dram_tensor("w", [n, k], F32, kind="ExternalInput")
    b_dram = nc.dram_tensor("bias", [n], F32, kind="ExternalInput")
    out_dram = nc.dram_tensor("out_t", [n, m_loc], F32, kind="ExternalOutput")

    M1 = 6291456.0        # 0.75 * 2^23  (magic for quantum 0.5)
    M2 = 12582912.0       # 0.75 * 2^24  (magic for quantum 1)
    DEK = 1048577.0       # 2^20 + 1     (Dekker split to 4 sig bits)
    SUBM = 24576.0        # 1.5 * 2^14   (magic for quantum 2^-9)
    THR = 0.015625        # 2^-6         (fp8 normal/subnormal boundary)
    SIXTH = float(np.float32(1.0) / np.float32(6.0))

    with tile.TileContext(nc) as tc:
        with (
            tc.tile_pool(name="big_f32", bufs=2) as bigf,       # x/w fp32 tiles
            tc.tile_pool(name="q_bf16", bufs=2) as qpool,       # e2m1 values
            tc.tile_pool(name="magic", bufs=2) as mpool,        # magic constants
            tc.tile_pool(name="wqt", bufs=2) as tpool,          # transposed wq
            tc.tile_pool(name="smalls", bufs=2) as spool,       # per-group scalars
            tc.tile_pool(name="outs", bufs=2) as opool,         # eviction staging
            tc.tile_pool(name="singles", bufs=1) as singles,    # xqT + bias
            tc.tile_pool(name="psum", bufs=2, space="PSUM") as ppool,
            tc.tile_pool(name="dram", bufs=1, space="DRAM") as dpool,
        ):
            # bias: [n] -> [128, NT] (partition-major within each n-tile)
            bias_sb = singles.tile([P, NT], F32)
            nc.scalar.dma_start(
                out=bias_sb[:],
                in_=b_dram[:].rearrange("(t p) -> p t", p=P),
            )

            # resident transposed quantized activations [128, KC, m_loc]
            xqT = singles.tile([P, KC, m_loc], BF16)
            dqx_dram = dpool.tile([m_loc, k], BF16)

            def quantize(src_tile):
                """src [128, k] f32 -> (q bf16 [128, k], s f32 [128, KG]).
                Overwrites src_tile in place with r = src / scale."""
                amax = spool.tile([P, KG], F32, tag="g1")
                nc.vector.tensor_reduce(
                    out=amax[:],
                    in_=src_tile[:].rearrange("p (g e) -> p g e", e=G),
                    axis=mybir.AxisListType.X,
                    op=mybir.AluOpType.max,
                    apply_absolute_value=True,
                )
                hi = spool.tile([P, KG], F32, tag="g2")
                nc.vector._custom_dve(dek_op, out=hi[:], in0=amax[:],
                                      s0=SIXTH, s1=DEK)
                s = spool.tile([P, KG], F32, tag="g3")
                nc.vector._custom_dve(ssub_op, out=s[:], in0=amax[:],
                                      in1=hi[:], s0=SIXTH, s1=SUBM, imm2=THR)
                nr = spool.tile([P, KG], F32, tag="g1")
                scr = spool.tile([P, KG], F32, tag="g2")
                nc.vector.reciprocal_approx_accurate(out=nr[:], in_=s[:],
                                                     scratch=scr[:])
                # r = g * (1/s), in place over the fp32 source tile (gpsimd)
                nr_b = nr[:].unsqueeze(2).broadcast_to([P, KG, G])
                nc.gpsimd.tensor_tensor(
                    out=src_tile[:].rearrange("p (g e) -> p g e", e=G),
                    in0=src_tile[:].rearrange("p (g e) -> p g e", e=G),
                    in1=nr_b,
                    op=mybir.AluOpType.mult,
                )
                # m = M1 * (1 + (r^2>=4) + 2*(r^2>=16))
                mt = mpool.tile([P, k], BF16)
                nc.vector._custom_dve(magic_op, out=mt[:], in0=src_tile[:],
                                      s0=4.0, s1=M1, imm2=M2)
                # q = (clip(r,-6,6) + m) - m
                qt = qpool.tile([P, k], BF16, tag="qvals")
                nc.vector._custom_dve(quant_op, out=qt[:], in0=src_tile[:],
                                      in1=mt[:], s0=6.0, s1=-6.0)
                return qt, s

            def dequant_inplace(qt, s):
                s_b = s[:].unsqueeze(2).broadcast_to([P, KG, G])
                nc.gpsimd.tensor_tensor(
                    out=qt[:].rearrange("p (g e) -> p g e", e=G),
                    in0=qt[:].rearrange("p (g e) -> p g e", e=G),
                    in1=s_b,
                    op=mybir.AluOpType.mult,
                )

            # ---- phase A: quantize x shard, spill dq_x to DRAM ----
            for mt in range(MT):
                x_t = bigf.tile([P, k], F32)
                nc.scalar.dma_start(out=x_t[:], in_=x_dram[mt * P:(mt + 1) * P, :])
                qt, s = quantize(x_t)
                dequant_inplace(qt, s)
                nc.scalar.dma_start(out=dqx_dram[mt * P:(mt + 1) * P, :],
                                    in_=qt[:])

            # ---- phase B: transpose dq_x into resident xqT ----
            for kc in range(KC):
                nc.sync.dma_start(
                    out=xqT[:, kc, :],
                    in_=dqx_dram[:, kc * P:(kc + 1) * P],
                    transpose=True,
                )

            # ---- phase C: per n-tile: quantize w, transpose, matmul ----
            for nt in range(NT):
                w_t = bigf.tile([P, k], F32)
                nc.scalar.dma_start(out=w_t[:], in_=w_dram[nt * P:(nt + 1) * P, :])
                qt, s = quantize(w_t)
                dequant_inplace(qt, s)

                wqT = tpool.tile([P, KC, P], BF16)
                for kc in range(KC):
                    nc.sync.dma_start(
                        out=wqT[:, kc, :],
                        in_=qt[:, kc * P:(kc + 1) * P],
                        transpose=True,
                    )

                psum = ppool.tile([P, m_loc], F32)
                for kc in range(KC):
                    for mh in range(MH):
                        nc.tensor.matmul(
                            out=psum[:, mh * 512:(mh + 1) * 512],
                            lhsT=wqT[:, kc, :],
                            rhs=xqT[:, kc, mh * 512:(mh + 1) * 512],
                            start=(kc == 0),
                            stop=(kc == KC - 1),
                        )

                o_t = opool.tile([P, m_loc], F32)
                nc.scalar.add(out=o_t[:], in_=psum[:], add=bias_sb[:, nt:nt + 1])
                nc.scalar.dma_start(out=out_dram[nt * P:(nt + 1) * P, :],
                                    in_=o_t[:])

    nc.compile()
    return nc


def _get_program(key=(M_LOC, K_FULL, N_FULL)):
    if key not in _PROGRAM_CACHE:
        _PROGRAM_CACHE[key] = build_program(*key)
    return _PROGRAM_CACHE[key]


def kernel(x, weight, bias, _trace=False):
    from concourse import bass_utils

    x = np.ascontiguousarray(np.asarray(x, dtype=np.float32))
    weight = np.ascontiguousarray(np.asarray(weight, dtype=np.float32))
    bias = np.ascontiguousarray(np.asarray(bias, dtype=np.float32))
    assert x.shape == (M_FULL, K_FULL) and weight.shape == (N_FULL, K_FULL)

    nc = _get_program()

    in_maps = [
        {
            "x_shard": x[c * M_LOC:(c + 1) * M_LOC, :],
            "w": weight,
            "bias": bias,
        }
        for c in range(N_CORES)
    ]
    res = bass_utils.run_bass_kernel_spmd(
        nc, in_maps, core_ids=list(range(N_CORES)), trace=_trace,
    )

    out = np.empty((M_FULL, N_FULL), dtype=np.float32)
    for c in range(N_CORES):
        out[c * M_LOC:(c + 1) * M_LOC, :] = res.results[c]["out_t"].T
    if _trace:
        kernel._last_results = res
    return out
